# revision 35
# baseline (speedup 1.0000x reference)
"""Trainium2 Bass kernel for octonion causal self-attention (fp8 projections).

Sharding: 8 cores = 4 batches x 2 head-groups. Core c handles batch b=c//2 and
head-group g=c%2 (octonion output components 4g..4g+3 = heads 8g..8g+7).
Each core computes q/k/v projections for its components from the full x[b],
RoPE, causal attention for its 8 heads, the octonion head-mixer for its group,
and a partial wo projection (its 4 input components, all 2048 output channels).
The host sums the two bf16 partials per batch and transposes. No collectives.

Projections exploit the ternary weights: the {-1,0,1} pattern is exact in
fp8e4, so Q/K/V run as fp8 DoubleRow matmuls (K=256 per instruction, 2x bf16
PE throughput, measured). Per-matrix ternary scales are approximated by the
tensor mean (~0.3% err) and folded away for free: s_q*s_k/sqrt(D) into the
rope cos/sin tables, s_v into the diag(1/l) identity. x ships as fp8 hi+lo
planes: Q/K use hi only (quantization noise averages out over the q.k
contraction; measured +0.06% err), V accumulates both planes in one PSUM
group for ~1e-3 exactness (v noise would pass straight through to the
output). Attention and the mixer-fused wo stay bf16 (wo_fused = M^T Wo is
dense, and as the last layer it tolerates no fp8 noise). PSUM stays f32.

Causal structure: S matmuls, exp and PV are trimmed to the lower triangle at
128-column granularity; the causal mask is a single 128x128 ident-stationary
matmul accumulated onto the diagonal score block. Probs transposes are
regular matmuls whose moving operand is diag(s_v/l), folding softmax
normalization into the transpose. Attention runs as one 16-step (t-chunk,
head) pipeline, 2 deep, with qc0's fused wo interleaved into later steps.

NOTE (measured on HW): keep exactly ONE open accumulation group per PSUM
bank. Narrow (128-col) matmul chunks with interleaved starts across regions
of one bank silently corrupt results. Wide 512-col groups as written are
correct; per-column PE cost is ~0.52ns (bf16) / ~0.51ns per 256-K-equiv (DR).
Output stores split across the sync (qc0) and scalar (qc1) HWDGE rings so
the next body's x_hi/weights (gpsimd SWDGE ring) aren't queued behind them.
"""

import math
import os
from contextlib import ExitStack

import numpy as np

B, T, C, H, D = 4, 1024, 2048, 16, 128
C8 = C // 8  # 256
NCORES = 8
P = 128
NEGM = -30000.0


# ---------------- octonion tables (matches reference) ----------------
def _cd_conj(a):
    n = a.shape[0]
    if n == 1:
        return a
    h = n // 2
    return np.concatenate([_cd_conj(a[:h]), -a[h:]])


def _cd_mul(a, b):
    n = a.shape[0]
    if n == 1:
        return a * b
    h = n // 2
    a1, a2 = a[:h], a[h:]
    c1, c2 = b[:h], b[h:]
    return np.concatenate(
        [
            _cd_mul(a1, c1) - _cd_mul(_cd_conj(c2), a2),
            _cd_mul(c2, a1) + _cd_mul(a2, _cd_conj(c1)),
        ]
    )


def _octonion_tables():
    signs = np.zeros((8, 8), dtype=np.float32)
    widx = np.zeros((8, 8), dtype=np.int32)
    for i in range(8):
        for j in range(8):
            ei = np.zeros(8)
            ei[i] = 1.0
            ej = np.zeros(8)
            ej[j] = 1.0
            p = _cd_mul(ei, ej)
            k = int(np.argmax(np.abs(p)))
            signs[i, j] = np.sign(p[k])
            widx[i, j] = k
    return signs, widx


SIGNS, WIDX = _octonion_tables()

_EVENS_FIRST = np.concatenate([np.arange(0, D, 2), np.arange(1, D, 2)])


def _bf16(a):
    import ml_dtypes

    return np.asarray(a, dtype=np.float32).astype(ml_dtypes.bfloat16)


def _ternary_quantize(W: np.ndarray) -> np.ndarray:
    """Replicates reference ternary_ste forward pass bit-exactly (jnp on CPU)."""
    import jax
    import jax.numpy as jnp

    with jax.default_device(jax.devices("cpu")[0]):
        Wj = jnp.asarray(W)
        s = jnp.mean(jnp.abs(Wj), axis=(-2, -1), keepdims=True) + 1e-8
        Wq = jnp.clip(jnp.round(Wj / s), -1.0, 1.0) * s
        return np.asarray(Wq)


def _ternary_pattern_scales(W: np.ndarray):
    """Exact ternary pattern in {-1,0,1} (f32) + per-matrix scales s_k.

    Wq[k] == pattern[k] * s[k] bit-exactly up to the f32 product; the kernel
    uses pattern in fp8 (exact) and folds a common mean scale elsewhere."""
    import jax
    import jax.numpy as jnp

    with jax.default_device(jax.devices("cpu")[0]):
        Wj = jnp.asarray(W)
        s = jnp.mean(jnp.abs(Wj), axis=(-2, -1), keepdims=True) + 1e-8
        pat = jnp.clip(jnp.round(Wj / s), -1.0, 1.0)
        return np.asarray(pat), np.asarray(s).reshape(-1)


def _fp8(a):
    import ml_dtypes

    return np.asarray(a, dtype=np.float32).astype(ml_dtypes.float8_e4m3)


def _fp8_hi_lo(a):
    """Split f32 array into fp8 value + fp8 residual (sum ≈ a to ~1e-3 rel)."""
    hi = _fp8(a)
    lo = _fp8(np.asarray(a, dtype=np.float32) - hi.astype(np.float32))
    return hi, lo


def _signed_full(Wq: np.ndarray, i: int) -> np.ndarray:
    """[2048, 256] block column for octonion output component i:
    rows j*256:(j+1)*256 = SIGNS[i,j] * Wq[i^j]."""
    out = np.empty((C, C8), dtype=np.float32)
    for j in range(8):
        out[j * C8 : (j + 1) * C8, :] = SIGNS[i, j] * Wq[i ^ j]
    return out


def _pat_scale(Wq: np.ndarray):
    """Recover {-1,0,1} pattern and per-matrix scale from quantized ternary."""
    s = np.max(np.abs(Wq), axis=(1, 2))
    s = np.maximum(s, 1e-30)
    pat = np.round(Wq / s[:, None, None]).astype(np.float32)
    return pat, s


def _prep_core_inputs(inputs: dict, b: int, g: int, wq_q, wk_q, wv_q, wo_q):
    x = inputs["x"]
    fc, fs = inputs["freqs_cos"], inputs["freqs_sin"]
    mixer_W, mixer_beta = inputs["mixer_W"], inputs["mixer_beta"]

    wq_p, s_q = _pat_scale(wq_q)
    wk_p, s_k = _pat_scale(wk_q)
    wv_p, s_v = _pat_scale(wv_q)
    sq_bar, sk_bar, sv_bar = float(np.mean(s_q)), float(np.mean(s_k)), float(
        np.mean(s_v)
    )

    m = {}
    # x transposed, partition-major [p, ct, t]; fp8 hi plane feeds Q/K
    # (noise averages out over the q.k contraction), hi+lo feed V (exact)
    xTf = np.ascontiguousarray(x[b].T).reshape(16, P, T).transpose(1, 0, 2)
    m["x_hi"], m["x_lo"] = _fp8_hi_lo(xTf)

    # q/k ternary patterns (+-1, fp8-exact): [qk, li, dh, c_p, ct, d],
    # interleaved rope layout; scales folded into the rope tables below.
    wqk = np.empty((2, 4, 2, P, 16, P), dtype=np.float32)
    for qk, Wp in enumerate((wq_p, wk_p)):
        for li in range(4):
            i = 4 * g + li
            Bf = _signed_full(Wp, i)  # [2048, 256]
            for dh in range(2):
                Bh = Bf[:, dh * D : (dh + 1) * D]  # [2048, 128]
                wqk[qk, li, dh] = Bh[:, _EVENS_FIRST].reshape(16, P, P).transpose(
                    1, 0, 2
                )
    m["wqk"] = _fp8(wqk)

    # v ternary pattern: [lp, ct, c_p, dcol]; s_v folded into sident below
    wv = np.empty((2, 16, P, 512), dtype=np.float32)
    for lp in range(2):
        B2 = np.concatenate(
            [_signed_full(wv_p, 4 * g + 2 * lp + u) for u in range(2)], axis=1
        )  # [2048, 512]
        wv[lp] = B2.reshape(16, P, 512)
    m["wv"] = _fp8(wv.transpose(2, 0, 1, 3))  # [p, lp, ct, d]

    # wo with the head-mixer fused in: both are per-token linear maps on the
    # local 1024-dim feature space (z = M y, out = Wo^T z), so Wo_fused =
    # M^T Wo folds the mixer away entirely.  Local z/y channel = head*128+dim.
    wo = np.empty((16, P, 8, P), dtype=np.float32)
    for ft in range(16):
        i_o, fh = ft // 2, ft % 2
        for kt in range(8):
            j = 4 * g + kt // 2
            dloc = kt % 2
            blk = SIGNS[i_o, j] * wo_q[i_o ^ j]  # [256, 256]
            wo[ft, :, kt, :] = blk[dloc * P : (dloc + 1) * P, fh * P : (fh + 1) * P]
    A = wo.transpose(2, 1, 0, 3).reshape(1024, 2048)  # [(kt,p) z-chan, f]
    M = np.zeros((1024, 1024), dtype=np.float32)  # [z (i,e), y (j,d)]
    for i in range(8):
        for j in range(8):
            M[i * P : (i + 1) * P, j * P : (j + 1) * P] = (
                SIGNS[i, j] * mixer_W[i ^ j] * mixer_beta[None, :]
            ).T
    Af = M.T @ A  # [(j,d) y-chan, f]
    m["wo"] = _bf16(
        Af.reshape(8, P, 16, P).transpose(1, 2, 0, 3)
    )  # [p, ft, kt, f]

    # evens-first rope layout: rows 0..63 = even dims, 64..127 = odd dims.
    # rope(q')[p] = q'[p]*cosd[p] + q'[p xor 64]*sind[p]; swap = half exchange
    # gamma folds s_q*s_k*(1/sqrt(D)) into the shared q/k rope tables.
    gamma = math.sqrt(sq_bar * sk_bar / math.sqrt(D))
    cosP = np.ascontiguousarray(fc.T) * gamma  # [64, 1024]
    sinP = np.ascontiguousarray(fs.T) * gamma
    m["cosd"] = _bf16(np.concatenate([cosP, cosP], axis=0))
    m["sind"] = _bf16(np.concatenate([-sinP, sinP], axis=0))
    m["ident"] = _bf16(np.eye(P, dtype=np.float32))
    # s_v-scaled identity: folds the v scale into diag(1/l)
    m["sident"] = _bf16(np.eye(P, dtype=np.float32) * sv_bar)

    # causal mask for the 128x128 diagonal block: allow col j <= row p
    pidx = np.arange(P)[:, None]
    jidx = np.arange(P)[None, :]
    m["tri"] = _bf16(np.where(jidx <= pidx, 0.0, NEGM).astype(np.float32))
    return m


# ---------------- device program ----------------
_NC_CACHE = {}


def _build_nc(repeat: int = 1, pp_bufs: int = 2, attn_depth: int = 2):
    key = (repeat, pp_bufs, attn_depth)
    if key in _NC_CACHE:
        return _NC_CACHE[key]

    import concourse.mybir as mybir
    import concourse.tile as tile
    from concourse import bacc

    dt = mybir.dt
    ALU = mybir.AluOpType
    AF = mybir.ActivationFunctionType
    f32, bf16, f8 = dt.float32, dt.bfloat16, dt.float8e4
    DR = mybir.MatmulPerfMode.DoubleRow

    nc = bacc.Bacc("TRN2", target_bir_lowering=False)

    x_hi = nc.declare_dram_parameter("x_hi", [P, 16, T], f8, isOutput=False)
    x_lo = nc.declare_dram_parameter("x_lo", [P, 16, T], f8, isOutput=False)
    wqk = nc.declare_dram_parameter("wqk", [2, 4, 2, P, 16, P], f8, isOutput=False)
    wv = nc.declare_dram_parameter("wv", [P, 2, 16, 512], f8, isOutput=False)
    wo = nc.declare_dram_parameter("wo", [P, 16, 8, P], bf16, isOutput=False)
    cosd = nc.declare_dram_parameter("cosd", [P, T], bf16, isOutput=False)
    sind = nc.declare_dram_parameter("sind", [P, T], bf16, isOutput=False)
    trip = nc.declare_dram_parameter("tri", [P, P], bf16, isOutput=False)
    identp = nc.declare_dram_parameter("ident", [P, P], bf16, isOutput=False)
    sidentp = nc.declare_dram_parameter("sident", [P, P], bf16, isOutput=False)
    outT = nc.declare_dram_parameter("outT", [C, T], bf16, isOutput=True)

    with tile.TileContext(nc) as tc, ExitStack() as ctx:
        cst = ctx.enter_context(tc.tile_pool(name="cst", bufs=1))
        statp = ctx.enter_context(tc.tile_pool(name="statp", bufs=16))
        xp = ctx.enter_context(tc.tile_pool(name="xp", bufs=1))
        wqk_pool = ctx.enter_context(tc.tile_pool(name="wqkp", bufs=2))
        ropeA = ctx.enter_context(tc.tile_pool(name="ropeA", bufs=3))
        ropeB = ctx.enter_context(tc.tile_pool(name="ropeB", bufs=2))
        qks = ctx.enter_context(tc.tile_pool(name="qks", bufs=1))
        vsb = ctx.enter_context(tc.tile_pool(name="vsb", bufs=1))
        # PSUM pools: pst+psy always open (4 banks); pp (proj, 4 banks) and
        # pss (S, [128,1024] = 2 banks x 2 bufs) are phase-local.
        ps_big = ctx.enter_context(tc.tile_pool(name="psbig", bufs=2, space="PSUM"))
        ps_t = ctx.enter_context(tc.tile_pool(name="pst", bufs=2, space="PSUM"))
        ps_y = ctx.enter_context(tc.tile_pool(name="psy", bufs=2, space="PSUM"))

        # constants loaded once (gpsimd SWDGE queue)
        ident = cst.tile([P, P], bf16, tag="ident")
        nc.gpsimd.dma_start(ident[:], identp[:])
        cos_sb = cst.tile([P, T], bf16, tag="cos")
        nc.gpsimd.dma_start(cos_sb[:], cosd[:])
        sin_sb = cst.tile([P, T], bf16, tag="sin")
        nc.gpsimd.dma_start(sin_sb[:], sind[:])
        tri_sb = cst.tile([P, P], bf16, tag="tri")
        nc.gpsimd.dma_start(tri_sb[:], trip[:])
        sident_sb = cst.tile([P, P], bf16, tag="sident")
        nc.gpsimd.dma_start(sident_sb[:], sidentp[:])

        for _rep in range(repeat):
            qT_h = [qks.tile([P, T], bf16, tag=f"qT{i}", name=f"qTh{i}") for i in range(8)]
            kT_h = [qks.tile([P, T], bf16, tag=f"kT{i}", name=f"kTh{i}") for i in range(8)]
            v_t = [
                [vsb.tile([P, 512], bf16, tag=f"v{lp}_{tt}", name=f"vt{lp}_{tt}") for tt in range(8)]
                for lp in range(2)
            ]
            wv_cm = tc.tile_pool(name="wvp", bufs=1)
            wv_pool = wv_cm.__enter__()


            # prefetch the first two q-weight tiles ahead of the big x loads
            # so the first matmul's stationary tile isn't queued behind 2 MB
            pre_wt = {}
            for pf in range(2):
                wt = wqk_pool.tile([P, 16, P], f8, tag="wqk", name=f"wtp{pf}")
                nc.gpsimd.dma_start(wt[:], wqk[0, 0, pf])
                pre_wt[(0, 0, pf)] = wt

            # batched x + wv loads; x_hi + first weights ride the gpsimd
            # SWDGE ring (idle at the repeat boundary, unlike sync/scalar
            # which still carry the previous body's output stores).
            xh_all = xp.tile([P, 16, T], f8, tag="xhall", name="xhall")
            nc.gpsimd.dma_start(xh_all[:, 0:8, :], x_hi[:, 0:8, :])
            nc.gpsimd.dma_start(xh_all[:, 8:16, :], x_hi[:, 8:16, :])
            xl_all = xp.tile([P, 16, T], f8, tag="xlall", name="xlall")
            nc.scalar.dma_start(xl_all[:, 0:8, :], x_lo[:, 0:8, :])
            nc.scalar.dma_start(xl_all[:, 8:16, :], x_lo[:, 8:16, :])
            wv_all = wv_pool.tile([P, 2, 16, 512], f8, tag="wvall", name="wvall")
            nc.gpsimd.dma_start(wv_all[:, 0], wv[:, 0])
            nc.gpsimd.dma_start(wv_all[:, 1], wv[:, 1])

            # ---- Q/K projections with fused RoPE (DMA half-swap,
            # pipelined one iteration) ----
            def emit_rope(dest, hh, qsb):
                qsw = ropeB.tile([P, T], bf16, tag="qsw")
                nc.sync.dma_start(qsw[0:64, :], qsb[64:128, :])
                nc.scalar.dma_start(qsw[64:128, :], qsb[0:64, :])
                t1 = ropeB.tile([P, T], bf16, tag="t1")
                t2 = ropeB.tile([P, T], bf16, tag="t2")
                nc.vector.tensor_tensor(t1[:], qsb[:], cos_sb[:], ALU.mult)
                nc.vector.tensor_tensor(t2[:], qsw[:], sin_sb[:], ALU.mult)
                nc.vector.tensor_tensor(dest[hh][:], t1[:], t2[:], ALU.add)

            rope_box = [None]

            def emit_qk(qk, li, dh, inline_rope=False):
                dest_h = qT_h if qk == 0 else kT_h
                hh = li * 2 + dh
                if (qk, li, dh) in pre_wt:
                    wt = pre_wt.pop((qk, li, dh))
                else:
                    # paced loads (WAR on this body's own matmul progress) go
                    # on the HWDGE rings so they never head-of-line block the
                    # gpsimd ring that carries the next body's bulk loads
                    wt = wqk_pool.tile([P, 16, P], f8, tag="wqk")
                    eng = nc.sync if (hh % 2 == 0) else nc.gpsimd
                    eng.dma_start(wt[:], wqk[qk, li, dh])
                pps = ps_big.tile([P, T], f32, tag="big")
                # single fp8 plane: x-quant noise averages out over the
                # 128-dim q.k contraction (probs err ~0.9% rms)
                # cp outer / tci inner: alternating the two PSUM banks per
                # instruction measures ~12us/body FASTER in-kernel than
                # finishing one bank's accumulation before the next (flip
                # tested 279.2us vs 266.6us), opposite of the isolated
                # microbench -- keep this order.
                for cp in range(8):
                    for tci in range(2):
                        nc.tensor.matmul(
                            pps[:, tci * 512 : (tci + 1) * 512],
                            wt[:, 2 * cp : 2 * cp + 2, :],
                            xh_all[:, 2 * cp : 2 * cp + 2, tci * 512 : (tci + 1) * 512],
                            start=(cp == 0),
                            stop=(cp == 7),
                            perf_mode=DR,
                        )
                qsb = ropeA.tile([P, T], bf16, tag="qsb")
                nc.vector.tensor_copy(out=qsb[:], in_=pps[:])
                if rope_box[0] is not None:
                    emit_rope(*rope_box[0])
                rope_box[0] = (dest_h, hh, qsb)
                if inline_rope:
                    emit_rope(*rope_box[0])
                    rope_box[0] = None

            qk_iters = [
                (qk, li, dh) for qk in range(2) for li in range(4) for dh in range(2)
            ]
            for it in qk_iters:
                emit_qk(*it)
            emit_rope(*rope_box[0])
            rope_box[0] = None
            qk_queue = []

            # ---- V projection (stays in SBUF, single fp8 plane) ----
            for lp in range(2):
                for tt in range(8):
                    vps = ps_big.tile([P, T], f32, tag="big", name=f"vp{lp}_{tt}")
                    for pi, plane in enumerate((xh_all, xl_all)):
                        for cp in range(8):
                            nc.tensor.matmul(
                                vps[:, :512],
                                plane[:, 2 * cp : 2 * cp + 2, tt * P : (tt + 1) * P],
                                wv_all[:, lp, 2 * cp : 2 * cp + 2, :],
                                start=(pi == 0 and cp == 0),
                                stop=(pi == 1 and cp == 7),
                                perf_mode=DR,
                            )
                    nc.vector.tensor_copy(out=v_t[lp][tt][:], in_=vps[:, :512])

            wv_cm.__exit__(None, None, None)

            # attention-phase pools
            psb_cm = tc.tile_pool(name="psb", bufs=13)
            psb = psb_cm.__enter__()
            diag_cm = tc.tile_pool(name="diagp", bufs=13)
            diagp = diag_cm.__enter__()
            pt_cm = tc.tile_pool(name="ptsb", bufs=10)
            ptsb = pt_cm.__enter__()
            y_cm = tc.tile_pool(name="ysb", bufs=2)
            yp = y_cm.__enter__()
            wo_cm = tc.tile_pool(name="wop", bufs=1)
            wop = wo_cm.__enter__()
            out_cm = tc.tile_pool(name="outp", bufs=2)
            outp = out_cm.__enter__()

            wo_all = wop.tile([P, 16, 8, P], bf16, tag="woall", name="woall")
            nc.gpsimd.dma_start(wo_all[:, 0:8], wo[:, 0:8])
            nc.gpsimd.dma_start(wo_all[:, 8:16], wo[:, 8:16])
            wo_t = [wo_all[:, ft] for ft in range(16)]

            def emit_wo_ft(ft, z_src, tsl_prev, osb_box, out_eng):
                if ft % 4 == 0:
                    osb_box[0] = outp.tile(
                        [P, 4, 512], bf16, tag="osb", name=f"osbd{ft}_{_rep}"
                    )
                osb = osb_box[0]
                ops = ps_t.tile([P, 512], f32, tag="tp")
                for kt in range(8):
                    nc.tensor.matmul(
                        ops[:],
                        wo_t[ft][:, kt, :],
                        z_src[:, kt, :],
                        start=(kt == 0),
                        stop=(kt == 7),
                    )
                nc.vector.tensor_copy(out=osb[:, ft % 4, :], in_=ops[:])
                if ft % 4 == 3:
                    f0 = ft - 3
                    out_eng.dma_start(
                        outT[f0 * P : (f0 + 4) * P, tsl_prev].rearrange(
                            "(f p) t -> p f t", p=P
                        ),
                        osb[:],
                    )

            def emit_S(h, qc):
                """S matmuls + diag mask + exp + recip + diag(r) for 4 q-blocks."""
                Ps_list = {}
                diag_list = {}
                lt_all = statp.tile([P, 4], f32, tag="l")
                rec_all = statp.tile([P, 4], f32, tag="r")
                for qt in range(4 * qc, 4 * qc + 4):
                    wq_w = (qt + 1) * P
                    Ps = psb.tile([P, T], bf16, tag="P", name=f"Ps{qc}_{h}_{qt}")
                    Ps_list[qt] = Ps
                    sps = ps_big.tile([P, T], f32, tag="big", name=f"sps{qc}_{h}_{qt}")
                    for chi in range((wq_w + 511) // 512):
                        w = min(512, wq_w - chi * 512)
                        nc.tensor.matmul(
                            sps[:, chi * 512 : chi * 512 + w],
                            qT_h[h][:, qt * P : (qt + 1) * P],
                            kT_h[h][:, chi * 512 : chi * 512 + w],
                            start=True,
                            stop=False,
                            skip_group_check=True,
                        )
                    nc.tensor.matmul(
                        sps[:, wq_w - P : wq_w],
                        ident[:],
                        tri_sb[:],
                        start=False,
                        stop=True,
                        skip_group_check=True,
                    )
                    nc.scalar.activation(
                        Ps[:, :wq_w],
                        sps[:, :wq_w],
                        AF.Exp,
                        accum_out=lt_all[:, qt % 4 : qt % 4 + 1],
                    )
                nc.vector.reciprocal(rec_all[:], lt_all[:])
                for qt in range(4 * qc, 4 * qc + 4):
                    dg = diagp.tile([P, P], bf16, tag="diag", name=f"dg{qc}_{h}_{qt}")
                    nc.vector.tensor_scalar(
                        dg[:], sident_sb[:], rec_all[:, qt % 4 : qt % 4 + 1], None,
                        op0=ALU.mult,
                    )
                    diag_list[qt] = dg
                return Ps_list, diag_list

            def emit_TPV(h, qc, Ps_list, diag_list, y_sb):
                """Normalizing transposes (regular matmuls vs diag(1/l)) + PV."""
                nkt = 4 * (qc + 1)
                pts = []
                for kt in range(nkt):
                    qt0 = max(kt, 4 * qc)
                    off = (qt0 - 4 * qc) * P
                    ptps = ps_t.tile([P, 512], f32, tag="tp")
                    for qt in range(qt0, 4 * qc + 4):
                        cl = (qt % 4) * P
                        nc.tensor.matmul(
                            ptps[:, cl : cl + P],
                            Ps_list[qt][:, kt * P : (kt + 1) * P],
                            diag_list[qt][:],
                            start=True,
                            stop=True,
                        )
                    pt_sb = ptsb.tile([P, 512], bf16, tag="PT", name=f"PT{qc}_{h}_{kt}")
                    nc.vector.tensor_copy(out=pt_sb[:, off:], in_=ptps[:, off:])
                    pts.append((pt_sb, off))
                yps = ps_y.tile([P, 512], f32, tag="y")
                lp, dcol = h // 4, (h % 4) * P
                for kt in range(nkt):
                    pt_sb, off = pts[kt]
                    nc.tensor.matmul(
                        yps[:, off:],
                        v_t[lp][kt][:, dcol : dcol + P],
                        pt_sb[:, off:],
                        start=(kt == 0),
                        stop=(kt == nkt - 1),
                        skip_group_check=True,
                    )
                nc.vector.tensor_copy(out=y_sb[:, h, :], in_=yps[:])

            # ---- one 16-step attention pipeline over (qc, h); qc0's wo
            # chunks flow in as soon as its heads complete ----
            y_sbs = {
                qc: yp.tile([P, 8, 512], bf16, tag="y", name=f"ysb{qc}")
                for qc in range(2)
            }
            tsls = {qc: slice(qc * 512, (qc + 1) * 512) for qc in range(2)}
            steps = [(qc, h) for qc in range(2) for h in range(8)]
            pending = []
            wo_queue = []
            osb_box = [None]
            for qc, h in steps:
                pending.append(((qc, h), emit_S(h, qc)))
                if qk_queue:
                    emit_qk(*qk_queue.pop(0), inline_rope=(len(qk_queue) == 0))
                for _ in range(3):
                    if wo_queue:
                        emit_wo_ft(wo_queue.pop(0), y_sbs[0], tsls[0], osb_box, nc.sync)
                if len(pending) > attn_depth:
                    (pqc, ph), cur = pending.pop(0)
                    emit_TPV(ph, pqc, cur[0], cur[1], y_sbs[pqc])
                    if (pqc, ph) == (0, 7):
                        wo_queue = list(range(16))
            for (pqc, ph), cur in pending:
                emit_TPV(ph, pqc, cur[0], cur[1], y_sbs[pqc])
            for ft in wo_queue:
                emit_wo_ft(ft, y_sbs[0], tsls[0], osb_box, nc.sync)
            box = [None]
            for ft in range(16):
                emit_wo_ft(ft, y_sbs[1], tsls[1], box, nc.scalar)

            out_cm.__exit__(None, None, None)
            wo_cm.__exit__(None, None, None)
            y_cm.__exit__(None, None, None)
            pt_cm.__exit__(None, None, None)
            diag_cm.__exit__(None, None, None)
            psb_cm.__exit__(None, None, None)

    nc.finalize()
    _NC_CACHE[key] = nc
    return nc


def _run(inputs: dict, trace: bool = False):
    from concourse.bass_utils import run_bass_kernel_spmd

    wq_q = _ternary_quantize(np.asarray(inputs["wq"], dtype=np.float32))
    wk_q = _ternary_quantize(np.asarray(inputs["wk"], dtype=np.float32))
    wv_q = _ternary_quantize(np.asarray(inputs["wv"], dtype=np.float32))
    wo_q = _ternary_quantize(np.asarray(inputs["wo"], dtype=np.float32))

    in_maps = []
    for c in range(NCORES):
        b, g = c // 2, c % 2
        in_maps.append(_prep_core_inputs(inputs, b, g, wq_q, wk_q, wv_q, wo_q))

    nc = _build_nc()
    res = run_bass_kernel_spmd(nc, in_maps, list(range(NCORES)), trace=trace)

    out = np.empty((B, T, C), dtype=np.float32)
    for b in range(B):
        acc = np.asarray(res.results[2 * b]["outT"]).astype(np.float32) + np.asarray(
            res.results[2 * b + 1]["outT"]
        ).astype(np.float32)
        out[b] = acc.T
    return out, res


def kernel(**inputs) -> np.ndarray:
    out, _ = _run(inputs, trace=False)
    return out



# revision 38
# speedup vs baseline: 1.0146x; 1.0146x over previous
"""Trainium2 Bass kernel for octonion causal self-attention (fp8 projections).

Sharding: 8 cores = 4 batches x 2 head-groups. Core c handles batch b=c//2 and
head-group g=c%2 (octonion output components 4g..4g+3 = heads 8g..8g+7).
Each core computes q/k/v projections for its components from the full x[b],
RoPE, causal attention for its 8 heads, the octonion head-mixer for its group,
and a partial wo projection (its 4 input components, all 2048 output channels).
The host sums the two bf16 partials per batch and transposes. No collectives.

Projections exploit the ternary weights: the {-1,0,1} pattern is exact in
fp8e4, so Q/K/V run as fp8 DoubleRow matmuls (K=256 per instruction, 2x bf16
PE throughput, measured). Per-matrix ternary scales are approximated by the
tensor mean (~0.3% err) and folded away for free: s_q*s_k/sqrt(D) into the
rope cos/sin tables, s_v into the diag(1/l) identity. x ships as fp8 hi+lo
planes: Q/K use hi only (quantization noise averages out over the q.k
contraction; measured +0.06% err), V accumulates both planes in one PSUM
group for ~1e-3 exactness (v noise would pass straight through to the
output). Attention and the mixer-fused wo stay bf16 (wo_fused = M^T Wo is
dense, and as the last layer it tolerates no fp8 noise). PSUM stays f32.

Causal structure: S matmuls, exp and PV are trimmed to the lower triangle at
128-column granularity; the causal mask is a single 128x128 ident-stationary
matmul accumulated onto the diagonal score block. Probs transposes are
regular matmuls whose moving operand is diag(s_v/l), folding softmax
normalization into the transpose. Attention runs as one 16-step (t-chunk,
head) pipeline, 2 deep, with qc0's fused wo interleaved into later steps.

NOTE (measured on HW): keep exactly ONE open accumulation group per PSUM
bank. Narrow (128-col) matmul chunks with interleaved starts across regions
of one bank silently corrupt results. Wide 512-col groups as written are
correct; per-column PE cost is ~0.52ns (bf16) / ~0.51ns per 256-K-equiv (DR).
Output stores split across the sync (qc0) and scalar (qc1) HWDGE rings so
the next body's x_hi/weights (gpsimd SWDGE ring) aren't queued behind them.
"""

import math
import os
from contextlib import ExitStack

import numpy as np

B, T, C, H, D = 4, 1024, 2048, 16, 128
C8 = C // 8  # 256
NCORES = 8
P = 128
NEGM = -30000.0


# ---------------- octonion tables (matches reference) ----------------
def _cd_conj(a):
    n = a.shape[0]
    if n == 1:
        return a
    h = n // 2
    return np.concatenate([_cd_conj(a[:h]), -a[h:]])


def _cd_mul(a, b):
    n = a.shape[0]
    if n == 1:
        return a * b
    h = n // 2
    a1, a2 = a[:h], a[h:]
    c1, c2 = b[:h], b[h:]
    return np.concatenate(
        [
            _cd_mul(a1, c1) - _cd_mul(_cd_conj(c2), a2),
            _cd_mul(c2, a1) + _cd_mul(a2, _cd_conj(c1)),
        ]
    )


def _octonion_tables():
    signs = np.zeros((8, 8), dtype=np.float32)
    widx = np.zeros((8, 8), dtype=np.int32)
    for i in range(8):
        for j in range(8):
            ei = np.zeros(8)
            ei[i] = 1.0
            ej = np.zeros(8)
            ej[j] = 1.0
            p = _cd_mul(ei, ej)
            k = int(np.argmax(np.abs(p)))
            signs[i, j] = np.sign(p[k])
            widx[i, j] = k
    return signs, widx


SIGNS, WIDX = _octonion_tables()

_EVENS_FIRST = np.concatenate([np.arange(0, D, 2), np.arange(1, D, 2)])


def _bf16(a):
    import ml_dtypes

    return np.asarray(a, dtype=np.float32).astype(ml_dtypes.bfloat16)


def _ternary_quantize(W: np.ndarray) -> np.ndarray:
    """Replicates reference ternary_ste forward pass bit-exactly (jnp on CPU)."""
    import jax
    import jax.numpy as jnp

    with jax.default_device(jax.devices("cpu")[0]):
        Wj = jnp.asarray(W)
        s = jnp.mean(jnp.abs(Wj), axis=(-2, -1), keepdims=True) + 1e-8
        Wq = jnp.clip(jnp.round(Wj / s), -1.0, 1.0) * s
        return np.asarray(Wq)


def _ternary_pattern_scales(W: np.ndarray):
    """Exact ternary pattern in {-1,0,1} (f32) + per-matrix scales s_k.

    Wq[k] == pattern[k] * s[k] bit-exactly up to the f32 product; the kernel
    uses pattern in fp8 (exact) and folds a common mean scale elsewhere."""
    import jax
    import jax.numpy as jnp

    with jax.default_device(jax.devices("cpu")[0]):
        Wj = jnp.asarray(W)
        s = jnp.mean(jnp.abs(Wj), axis=(-2, -1), keepdims=True) + 1e-8
        pat = jnp.clip(jnp.round(Wj / s), -1.0, 1.0)
        return np.asarray(pat), np.asarray(s).reshape(-1)


def _fp8(a):
    import ml_dtypes

    return np.asarray(a, dtype=np.float32).astype(ml_dtypes.float8_e4m3)


def _fp8_hi_lo(a):
    """Split f32 array into fp8 value + fp8 residual (sum ≈ a to ~1e-3 rel)."""
    hi = _fp8(a)
    lo = _fp8(np.asarray(a, dtype=np.float32) - hi.astype(np.float32))
    return hi, lo


def _signed_full(Wq: np.ndarray, i: int) -> np.ndarray:
    """[2048, 256] block column for octonion output component i:
    rows j*256:(j+1)*256 = SIGNS[i,j] * Wq[i^j]."""
    out = np.empty((C, C8), dtype=np.float32)
    for j in range(8):
        out[j * C8 : (j + 1) * C8, :] = SIGNS[i, j] * Wq[i ^ j]
    return out


def _pat_scale(Wq: np.ndarray):
    """Recover {-1,0,1} pattern and per-matrix scale from quantized ternary."""
    s = np.max(np.abs(Wq), axis=(1, 2))
    s = np.maximum(s, 1e-30)
    pat = np.round(Wq / s[:, None, None]).astype(np.float32)
    return pat, s


def _prep_core_inputs(inputs: dict, b: int, g: int, wq_q, wk_q, wv_q, wo_q):
    x = inputs["x"]
    fc, fs = inputs["freqs_cos"], inputs["freqs_sin"]
    mixer_W, mixer_beta = inputs["mixer_W"], inputs["mixer_beta"]

    wq_p, s_q = _pat_scale(wq_q)
    wk_p, s_k = _pat_scale(wk_q)
    wv_p, s_v = _pat_scale(wv_q)
    sq_bar, sk_bar, sv_bar = float(np.mean(s_q)), float(np.mean(s_k)), float(
        np.mean(s_v)
    )

    m = {}
    # x transposed, partition-major [p, ct, t]; fp8 hi plane feeds Q/K
    # (noise averages out over the q.k contraction), hi+lo feed V (exact)
    xTf = np.ascontiguousarray(x[b].T).reshape(16, P, T).transpose(1, 0, 2)
    m["x_hi"], m["x_lo"] = _fp8_hi_lo(xTf)

    # q/k ternary patterns (+-1, fp8-exact): [qk, li, dh, c_p, ct, d],
    # interleaved rope layout; scales folded into the rope tables below.
    wqk = np.empty((2, 4, 2, P, 16, P), dtype=np.float32)
    for qk, Wp in enumerate((wq_p, wk_p)):
        for li in range(4):
            i = 4 * g + li
            Bf = _signed_full(Wp, i)  # [2048, 256]
            for dh in range(2):
                Bh = Bf[:, dh * D : (dh + 1) * D]  # [2048, 128]
                wqk[qk, li, dh] = Bh[:, _EVENS_FIRST].reshape(16, P, P).transpose(
                    1, 0, 2
                )
    m["wqk"] = _fp8(wqk)

    # v ternary pattern: [lp, ct, c_p, dcol]; s_v folded into sident below
    wv = np.empty((2, 16, P, 512), dtype=np.float32)
    for lp in range(2):
        B2 = np.concatenate(
            [_signed_full(wv_p, 4 * g + 2 * lp + u) for u in range(2)], axis=1
        )  # [2048, 512]
        wv[lp] = B2.reshape(16, P, 512)
    m["wv"] = _fp8(wv.transpose(2, 0, 1, 3))  # [p, lp, ct, d]

    # wo with the head-mixer fused in: both are per-token linear maps on the
    # local 1024-dim feature space (z = M y, out = Wo^T z), so Wo_fused =
    # M^T Wo folds the mixer away entirely.  Local z/y channel = head*128+dim.
    wo = np.empty((16, P, 8, P), dtype=np.float32)
    for ft in range(16):
        i_o, fh = ft // 2, ft % 2
        for kt in range(8):
            j = 4 * g + kt // 2
            dloc = kt % 2
            blk = SIGNS[i_o, j] * wo_q[i_o ^ j]  # [256, 256]
            wo[ft, :, kt, :] = blk[dloc * P : (dloc + 1) * P, fh * P : (fh + 1) * P]
    A = wo.transpose(2, 1, 0, 3).reshape(1024, 2048)  # [(kt,p) z-chan, f]
    M = np.zeros((1024, 1024), dtype=np.float32)  # [z (i,e), y (j,d)]
    for i in range(8):
        for j in range(8):
            M[i * P : (i + 1) * P, j * P : (j + 1) * P] = (
                SIGNS[i, j] * mixer_W[i ^ j] * mixer_beta[None, :]
            ).T
    Af = M.T @ A  # [(j,d) y-chan, f]
    m["wo"] = _bf16(
        Af.reshape(8, P, 16, P).transpose(1, 2, 0, 3)
    )  # [p, ft, kt, f]

    # evens-first rope layout: rows 0..63 = even dims, 64..127 = odd dims.
    # rope(q')[p] = q'[p]*cosd[p] + q'[p xor 64]*sind[p]; swap = half exchange
    # gamma folds s_q*s_k*(1/sqrt(D)) into the shared q/k rope tables.
    gamma = math.sqrt(sq_bar * sk_bar / math.sqrt(D))
    cosP = np.ascontiguousarray(fc.T) * gamma  # [64, 1024]
    sinP = np.ascontiguousarray(fs.T) * gamma
    m["cosd"] = _bf16(np.concatenate([cosP, cosP], axis=0))
    m["sind"] = _bf16(np.concatenate([-sinP, sinP], axis=0))
    m["ident"] = _bf16(np.eye(P, dtype=np.float32))
    # s_v-scaled identity: folds the v scale into diag(1/l)
    m["sident"] = _bf16(np.eye(P, dtype=np.float32) * sv_bar)

    # causal mask for the 128x128 diagonal block: allow col j <= row p
    pidx = np.arange(P)[:, None]
    jidx = np.arange(P)[None, :]
    m["tri"] = _bf16(np.where(jidx <= pidx, 0.0, NEGM).astype(np.float32))
    return m


# ---------------- device program ----------------
_NC_CACHE = {}


def _build_nc(repeat: int = 1, pp_bufs: int = 2, attn_depth: int = 2):
    key = (repeat, pp_bufs, attn_depth)
    if key in _NC_CACHE:
        return _NC_CACHE[key]

    import concourse.mybir as mybir
    import concourse.tile as tile
    from concourse import bacc

    dt = mybir.dt
    ALU = mybir.AluOpType
    AF = mybir.ActivationFunctionType
    f32, bf16, f8 = dt.float32, dt.bfloat16, dt.float8e4
    DR = mybir.MatmulPerfMode.DoubleRow

    nc = bacc.Bacc("TRN2", target_bir_lowering=False)

    x_hi = nc.declare_dram_parameter("x_hi", [P, 16, T], f8, isOutput=False)
    x_lo = nc.declare_dram_parameter("x_lo", [P, 16, T], f8, isOutput=False)
    wqk = nc.declare_dram_parameter("wqk", [2, 4, 2, P, 16, P], f8, isOutput=False)
    wv = nc.declare_dram_parameter("wv", [P, 2, 16, 512], f8, isOutput=False)
    wo = nc.declare_dram_parameter("wo", [P, 16, 8, P], bf16, isOutput=False)
    cosd = nc.declare_dram_parameter("cosd", [P, T], bf16, isOutput=False)
    sind = nc.declare_dram_parameter("sind", [P, T], bf16, isOutput=False)
    trip = nc.declare_dram_parameter("tri", [P, P], bf16, isOutput=False)
    identp = nc.declare_dram_parameter("ident", [P, P], bf16, isOutput=False)
    sidentp = nc.declare_dram_parameter("sident", [P, P], bf16, isOutput=False)
    outT = nc.declare_dram_parameter("outT", [C, T], bf16, isOutput=True)

    with tile.TileContext(nc) as tc, ExitStack() as ctx:
        cst = ctx.enter_context(tc.tile_pool(name="cst", bufs=1))
        statp = ctx.enter_context(tc.tile_pool(name="statp", bufs=16))
        xp = ctx.enter_context(tc.tile_pool(name="xp", bufs=1))
        wqk_pool = ctx.enter_context(tc.tile_pool(name="wqkp", bufs=2))
        ropeA = ctx.enter_context(tc.tile_pool(name="ropeA", bufs=3))
        ropeB = ctx.enter_context(tc.tile_pool(name="ropeB", bufs=2))
        qks = ctx.enter_context(tc.tile_pool(name="qks", bufs=1))
        vsb = ctx.enter_context(tc.tile_pool(name="vsb", bufs=1))
        # PSUM pools: pst+psy always open (4 banks); pp (proj, 4 banks) and
        # pss (S, [128,1024] = 2 banks x 2 bufs) are phase-local.
        ps_big = ctx.enter_context(tc.tile_pool(name="psbig", bufs=2, space="PSUM"))
        ps_t = ctx.enter_context(tc.tile_pool(name="pst", bufs=2, space="PSUM"))
        ps_y = ctx.enter_context(tc.tile_pool(name="psy", bufs=2, space="PSUM"))

        # constants loaded once (gpsimd SWDGE queue)
        ident = cst.tile([P, P], bf16, tag="ident")
        nc.gpsimd.dma_start(ident[:], identp[:])
        cos_sb = cst.tile([P, T], bf16, tag="cos")
        nc.gpsimd.dma_start(cos_sb[:], cosd[:])
        sin_sb = cst.tile([P, T], bf16, tag="sin")
        nc.gpsimd.dma_start(sin_sb[:], sind[:])
        tri_sb = cst.tile([P, P], bf16, tag="tri")
        nc.gpsimd.dma_start(tri_sb[:], trip[:])
        sident_sb = cst.tile([P, P], bf16, tag="sident")
        nc.gpsimd.dma_start(sident_sb[:], sidentp[:])

        for _rep in range(repeat):
            qT_h = [qks.tile([P, T], bf16, tag=f"qT{i}", name=f"qTh{i}") for i in range(8)]
            kT_h = [qks.tile([P, T], bf16, tag=f"kT{i}", name=f"kTh{i}") for i in range(8)]
            v_t = [
                [vsb.tile([P, 512], bf16, tag=f"v{lp}_{tt}", name=f"vt{lp}_{tt}") for tt in range(8)]
                for lp in range(2)
            ]
            wv_cm = tc.tile_pool(name="wvp", bufs=1)
            wv_pool = wv_cm.__enter__()


            # prefetch the first two q-weight tiles ahead of the big x loads
            # so the first matmul's stationary tile isn't queued behind 2 MB
            pre_wt = {}
            for pf in range(2):
                wt = wqk_pool.tile([P, 16, P], f8, tag="wqk", name=f"wtp{pf}")
                nc.gpsimd.dma_start(wt[:], wqk[0, 0, pf])
                pre_wt[(0, 0, pf)] = wt

            # batched x + wv loads; x_hi + first weights ride the gpsimd
            # SWDGE ring (idle at the repeat boundary, unlike sync/scalar
            # which still carry the previous body's output stores).
            xh_all = xp.tile([P, 16, T], f8, tag="xhall", name="xhall")
            nc.gpsimd.dma_start(xh_all[:, 0:8, :], x_hi[:, 0:8, :])
            nc.gpsimd.dma_start(xh_all[:, 8:16, :], x_hi[:, 8:16, :])
            xl_all = xp.tile([P, 16, T], f8, tag="xlall", name="xlall")
            nc.scalar.dma_start(xl_all[:, 0:8, :], x_lo[:, 0:8, :])
            nc.scalar.dma_start(xl_all[:, 8:16, :], x_lo[:, 8:16, :])
            wv_all = wv_pool.tile([P, 2, 16, 512], f8, tag="wvall", name="wvall")
            nc.gpsimd.dma_start(wv_all[:, 0], wv[:, 0])
            nc.gpsimd.dma_start(wv_all[:, 1], wv[:, 1])

            # ---- Q/K projections with fused RoPE (DMA half-swap,
            # pipelined one iteration) ----
            def emit_rope(dest, hh, qsb):
                qsw = ropeB.tile([P, T], bf16, tag="qsw")
                nc.sync.dma_start(qsw[0:64, :], qsb[64:128, :])
                nc.scalar.dma_start(qsw[64:128, :], qsb[0:64, :])
                t1 = ropeB.tile([P, T], bf16, tag="t1")
                t2 = ropeB.tile([P, T], bf16, tag="t2")
                nc.vector.tensor_tensor(t1[:], qsb[:], cos_sb[:], ALU.mult)
                nc.vector.tensor_tensor(t2[:], qsw[:], sin_sb[:], ALU.mult)
                nc.vector.tensor_tensor(dest[hh][:], t1[:], t2[:], ALU.add)

            rope_box = [None]

            def emit_qk(qk, li, dh, inline_rope=False):
                dest_h = qT_h if qk == 0 else kT_h
                hh = li * 2 + dh
                if (qk, li, dh) in pre_wt:
                    wt = pre_wt.pop((qk, li, dh))
                else:
                    # paced loads (WAR on this body's own matmul progress) go
                    # on the HWDGE rings so they never head-of-line block the
                    # gpsimd ring that carries the next body's bulk loads
                    wt = wqk_pool.tile([P, 16, P], f8, tag="wqk")
                    eng = nc.sync if (hh % 2 == 0) else nc.gpsimd
                    eng.dma_start(wt[:], wqk[qk, li, dh])
                pps = ps_big.tile([P, T], f32, tag="big")
                # single fp8 plane: x-quant noise averages out over the
                # 128-dim q.k contraction (probs err ~0.9% rms)
                # cp outer / tci inner: alternating the two PSUM banks per
                # instruction measures ~12us/body FASTER in-kernel than
                # finishing one bank's accumulation before the next (flip
                # tested 279.2us vs 266.6us), opposite of the isolated
                # microbench -- keep this order.
                for cp in range(8):
                    for tci in range(2):
                        nc.tensor.matmul(
                            pps[:, tci * 512 : (tci + 1) * 512],
                            wt[:, 2 * cp : 2 * cp + 2, :],
                            xh_all[:, 2 * cp : 2 * cp + 2, tci * 512 : (tci + 1) * 512],
                            start=(cp == 0),
                            stop=(cp == 7),
                            perf_mode=DR,
                        )
                qsb = ropeA.tile([P, T], bf16, tag="qsb")
                # PSUM->SBUF copy on the Activation engine (idle during proj;
                # DVE is saturated with the rope multiplies, and this copy
                # holds the pps PSUM buffer, pacing the next projection)
                nc.scalar.activation(qsb[:], pps[:], AF.Copy)
                if rope_box[0] is not None:
                    emit_rope(*rope_box[0])
                rope_box[0] = (dest_h, hh, qsb)
                if inline_rope:
                    emit_rope(*rope_box[0])
                    rope_box[0] = None

            qk_iters = [
                (qk, li, dh) for qk in range(2) for li in range(4) for dh in range(2)
            ]
            for it in qk_iters:
                emit_qk(*it)
            emit_rope(*rope_box[0])
            rope_box[0] = None
            qk_queue = []

            # ---- V projection (stays in SBUF, single fp8 plane) ----
            for lp in range(2):
                for tt in range(8):
                    vps = ps_big.tile([P, T], f32, tag="big", name=f"vp{lp}_{tt}")
                    for pi, plane in enumerate((xh_all, xl_all)):
                        for cp in range(8):
                            nc.tensor.matmul(
                                vps[:, :512],
                                plane[:, 2 * cp : 2 * cp + 2, tt * P : (tt + 1) * P],
                                wv_all[:, lp, 2 * cp : 2 * cp + 2, :],
                                start=(pi == 0 and cp == 0),
                                stop=(pi == 1 and cp == 7),
                                perf_mode=DR,
                            )
                    nc.vector.tensor_copy(out=v_t[lp][tt][:], in_=vps[:, :512])

            wv_cm.__exit__(None, None, None)

            # attention-phase pools
            psb_cm = tc.tile_pool(name="psb", bufs=13)
            psb = psb_cm.__enter__()
            diag_cm = tc.tile_pool(name="diagp", bufs=13)
            diagp = diag_cm.__enter__()
            pt_cm = tc.tile_pool(name="ptsb", bufs=10)
            ptsb = pt_cm.__enter__()
            y_cm = tc.tile_pool(name="ysb", bufs=2)
            yp = y_cm.__enter__()
            wo_cm = tc.tile_pool(name="wop", bufs=1)
            wop = wo_cm.__enter__()
            out_cm = tc.tile_pool(name="outp", bufs=2)
            outp = out_cm.__enter__()

            wo_all = wop.tile([P, 16, 8, P], bf16, tag="woall", name="woall")
            nc.gpsimd.dma_start(wo_all[:, 0:8], wo[:, 0:8])
            nc.gpsimd.dma_start(wo_all[:, 8:16], wo[:, 8:16])
            wo_t = [wo_all[:, ft] for ft in range(16)]

            def emit_wo_ft(ft, z_src, tsl_prev, osb_box, out_eng):
                if ft % 4 == 0:
                    osb_box[0] = outp.tile(
                        [P, 4, 512], bf16, tag="osb", name=f"osbd{ft}_{_rep}"
                    )
                osb = osb_box[0]
                ops = ps_t.tile([P, 512], f32, tag="tp")
                for kt in range(8):
                    nc.tensor.matmul(
                        ops[:],
                        wo_t[ft][:, kt, :],
                        z_src[:, kt, :],
                        start=(kt == 0),
                        stop=(kt == 7),
                    )
                nc.vector.tensor_copy(out=osb[:, ft % 4, :], in_=ops[:])
                if ft % 4 == 3:
                    f0 = ft - 3
                    out_eng.dma_start(
                        outT[f0 * P : (f0 + 4) * P, tsl_prev].rearrange(
                            "(f p) t -> p f t", p=P
                        ),
                        osb[:],
                    )

            def emit_S(h, qc):
                """S matmuls + diag mask + exp + recip + diag(r) for 4 q-blocks."""
                Ps_list = {}
                diag_list = {}
                lt_all = statp.tile([P, 4], f32, tag="l")
                rec_all = statp.tile([P, 4], f32, tag="r")
                for qt in range(4 * qc, 4 * qc + 4):
                    wq_w = (qt + 1) * P
                    Ps = psb.tile([P, T], bf16, tag="P", name=f"Ps{qc}_{h}_{qt}")
                    Ps_list[qt] = Ps
                    sps = ps_big.tile([P, T], f32, tag="big", name=f"sps{qc}_{h}_{qt}")
                    for chi in range((wq_w + 511) // 512):
                        w = min(512, wq_w - chi * 512)
                        nc.tensor.matmul(
                            sps[:, chi * 512 : chi * 512 + w],
                            qT_h[h][:, qt * P : (qt + 1) * P],
                            kT_h[h][:, chi * 512 : chi * 512 + w],
                            start=True,
                            stop=False,
                            skip_group_check=True,
                        )
                    nc.tensor.matmul(
                        sps[:, wq_w - P : wq_w],
                        ident[:],
                        tri_sb[:],
                        start=False,
                        stop=True,
                        skip_group_check=True,
                    )
                    nc.scalar.activation(
                        Ps[:, :wq_w],
                        sps[:, :wq_w],
                        AF.Exp,
                        accum_out=lt_all[:, qt % 4 : qt % 4 + 1],
                    )
                nc.vector.reciprocal(rec_all[:], lt_all[:])
                for qt in range(4 * qc, 4 * qc + 4):
                    dg = diagp.tile([P, P], bf16, tag="diag", name=f"dg{qc}_{h}_{qt}")
                    nc.vector.tensor_scalar(
                        dg[:], sident_sb[:], rec_all[:, qt % 4 : qt % 4 + 1], None,
                        op0=ALU.mult,
                    )
                    diag_list[qt] = dg
                return Ps_list, diag_list

            def emit_TPV(h, qc, Ps_list, diag_list, y_sb):
                """Normalizing transposes (regular matmuls vs diag(1/l)) + PV."""
                nkt = 4 * (qc + 1)
                pts = []
                for kt in range(nkt):
                    qt0 = max(kt, 4 * qc)
                    off = (qt0 - 4 * qc) * P
                    ptps = ps_t.tile([P, 512], f32, tag="tp")
                    for qt in range(qt0, 4 * qc + 4):
                        cl = (qt % 4) * P
                        nc.tensor.matmul(
                            ptps[:, cl : cl + P],
                            Ps_list[qt][:, kt * P : (kt + 1) * P],
                            diag_list[qt][:],
                            start=True,
                            stop=True,
                        )
                    pt_sb = ptsb.tile([P, 512], bf16, tag="PT", name=f"PT{qc}_{h}_{kt}")
                    nc.vector.tensor_copy(out=pt_sb[:, off:], in_=ptps[:, off:])
                    pts.append((pt_sb, off))
                yps = ps_y.tile([P, 512], f32, tag="y")
                lp, dcol = h // 4, (h % 4) * P
                for kt in range(nkt):
                    pt_sb, off = pts[kt]
                    nc.tensor.matmul(
                        yps[:, off:],
                        v_t[lp][kt][:, dcol : dcol + P],
                        pt_sb[:, off:],
                        start=(kt == 0),
                        stop=(kt == nkt - 1),
                        skip_group_check=True,
                    )
                nc.vector.tensor_copy(out=y_sb[:, h, :], in_=yps[:])

            # ---- one 16-step attention pipeline over (qc, h); qc0's wo
            # chunks flow in as soon as its heads complete ----
            y_sbs = {
                qc: yp.tile([P, 8, 512], bf16, tag="y", name=f"ysb{qc}")
                for qc in range(2)
            }
            tsls = {qc: slice(qc * 512, (qc + 1) * 512) for qc in range(2)}
            steps = [(qc, h) for qc in range(2) for h in range(8)]
            pending = []
            wo_queue = []
            osb_box = [None]
            for qc, h in steps:
                pending.append(((qc, h), emit_S(h, qc)))
                if qk_queue:
                    emit_qk(*qk_queue.pop(0), inline_rope=(len(qk_queue) == 0))
                for _ in range(3):
                    if wo_queue:
                        emit_wo_ft(wo_queue.pop(0), y_sbs[0], tsls[0], osb_box, nc.sync)
                if len(pending) > attn_depth:
                    (pqc, ph), cur = pending.pop(0)
                    emit_TPV(ph, pqc, cur[0], cur[1], y_sbs[pqc])
                    if (pqc, ph) == (0, 7):
                        wo_queue = list(range(16))
            for (pqc, ph), cur in pending:
                emit_TPV(ph, pqc, cur[0], cur[1], y_sbs[pqc])
            for ft in wo_queue:
                emit_wo_ft(ft, y_sbs[0], tsls[0], osb_box, nc.sync)
            box = [None]
            for ft in range(16):
                emit_wo_ft(ft, y_sbs[1], tsls[1], box, nc.scalar)

            out_cm.__exit__(None, None, None)
            wo_cm.__exit__(None, None, None)
            y_cm.__exit__(None, None, None)
            pt_cm.__exit__(None, None, None)
            diag_cm.__exit__(None, None, None)
            psb_cm.__exit__(None, None, None)

    nc.finalize()
    _NC_CACHE[key] = nc
    return nc


def _run(inputs: dict, trace: bool = False):
    from concourse.bass_utils import run_bass_kernel_spmd

    wq_q = _ternary_quantize(np.asarray(inputs["wq"], dtype=np.float32))
    wk_q = _ternary_quantize(np.asarray(inputs["wk"], dtype=np.float32))
    wv_q = _ternary_quantize(np.asarray(inputs["wv"], dtype=np.float32))
    wo_q = _ternary_quantize(np.asarray(inputs["wo"], dtype=np.float32))

    in_maps = []
    for c in range(NCORES):
        b, g = c // 2, c % 2
        in_maps.append(_prep_core_inputs(inputs, b, g, wq_q, wk_q, wv_q, wo_q))

    nc = _build_nc()
    res = run_bass_kernel_spmd(nc, in_maps, list(range(NCORES)), trace=trace)

    out = np.empty((B, T, C), dtype=np.float32)
    for b in range(B):
        acc = np.asarray(res.results[2 * b]["outT"]).astype(np.float32) + np.asarray(
            res.results[2 * b + 1]["outT"]
        ).astype(np.float32)
        out[b] = acc.T
    return out, res


def kernel(**inputs) -> np.ndarray:
    out, _ = _run(inputs, trace=False)
    return out

